# revision 2
# baseline (speedup 1.0000x reference)
"""GCN (2x GCNConv + MLP head + softmax) on 8 TRN2 NeuronCores.

Strategy (graph/data parallel, per sharding hint):
  - Nodes are sharded across 8 cores (2500 rows each, padded to 2560).
  - Weight matrices replicated.
  - Per layer: each core computes h = z @ W for its rows, pre-scales by
    dinv (deg^-1/2), AllGathers the scaled table (node-major), then
    aggregates messages for edges partitioned by dst (local windows of
    128 dst nodes) using dma_gather (row gather) + one-hot segment
    matmuls on the TensorEngine accumulating in PSUM. dinv[dst] is
    folded into the segment matrices host-side; self-loops are ordinary
    edges. Epilogue adds bias, applies relu, and PE-transposes into the
    feature-major layout the next matmul needs as lhsT.
  - Head: two dense layers + row softmax, all local.

Host-side preprocessing is limited to graph-structure work (edge sort,
degree counts, one-hot segment matrices, index layout) + sharding.
"""

import os
from contextlib import ExitStack

import numpy as np

import concourse.bacc as bacc
import concourse.mybir as mybir
import concourse.tile as tile
from concourse.bass_utils import run_bass_kernel_spmd
from concourse.masks import make_identity

# problem shapes (hardcoded per contract)
N = 20000
E = 320000
D = 512
D_OUT = 128
NCORES = 8
RPC = 2500          # real rows per core
RPAD = 2560         # padded rows per core (20 tiles of 128)
NPAD = RPAD * NCORES
MT = RPAD // 128    # m-tiles / dst windows per core (20)
G = 6               # max chunks (of 128 edges) per dma_gather call

# config
TABLE_DT = os.environ.get("GNN_TABLE_DT", "float32")   # "float32" | "bfloat16"
MM_F32R = os.environ.get("GNN_MM_F32R", "0") == "1"    # fast fp32 matmul mode

_f32 = mybir.dt.float32
_f32r = mybir.dt.float32r
_bf16 = mybir.dt.bfloat16
_i16 = mybir.dt.int16


def _pad_id(r):
    return r + 60 * (r // RPC)


def _prepare(x, edge_index, dinv):
    """Edge partitioning by dst + per-core S matrices and gather indices.

    Returns (per_core: list of dict, group_sizes: list[list[int]], TC).
    """
    src = np.concatenate([edge_index[0], np.arange(N, dtype=np.int64)])
    dst = np.concatenate([edge_index[1], np.arange(N, dtype=np.int64)])

    order = np.argsort(dst, kind="stable")
    srcs = src[order]
    dsts = dst[order]
    srcs_pad = _pad_id(srcs).astype(np.int64)
    dinv_dst = dinv[dsts]

    # per-(core, window) counts
    counts = np.zeros((NCORES, MT), dtype=np.int64)
    core_bounds = np.searchsorted(dsts, np.arange(NCORES + 1) * RPC)
    for c in range(NCORES):
        lo, hi = core_bounds[c], core_bounds[c + 1]
        d = dsts[lo:hi] - c * RPC
        wb = np.searchsorted(d, np.arange(MT + 1) * 128)
        counts[c] = wb[1:] - wb[:-1]

    cpw = np.maximum(1, -(-counts.max(axis=0) // 128))  # chunks per window
    TC = int(cpw.sum())
    chunk_base = np.concatenate([[0], np.cumsum(cpw)])[:-1]

    group_sizes = []
    for w in range(MT):
        n = int(cpw[w])
        gs = [G] * (n // G)
        if n % G:
            gs.append(n % G)
        group_sizes.append(gs)

    table_np = np.float32 if TABLE_DT == "float32" else None
    per_core = []
    for c in range(NCORES):
        S_np = np.zeros((TC, 128, 128), dtype=np.float32)
        gidx = np.zeros((TC, 128), dtype=np.int16)
        lo, hi = core_bounds[c], core_bounds[c + 1]
        d = dsts[lo:hi] - c * RPC
        s_ids = srcs_pad[lo:hi]
        dv = dinv_dst[lo:hi]
        wb = np.searchsorted(d, np.arange(MT + 1) * 128)
        for w in range(MT):
            a, b = wb[w], wb[w + 1]
            k = np.arange(b - a)
            tg = chunk_base[w] + (k // 128)
            row = k % 128
            S_np[tg, row, d[a:b] - w * 128] = dv[a:b]
            gidx[tg, row] = s_ids[a:b]
        # wrapped int16 index layout per gather call
        cols = []
        for w in range(MT):
            t0 = chunk_base[w]
            for gsz in group_sizes[w]:
                L = gidx[t0:t0 + gsz].reshape(-1)
                blk = L.reshape(-1, 16).T  # [16, nidx/16]
                cols.append(np.tile(blk, (8, 1)))
                t0 += gsz
        idx_np = np.ascontiguousarray(np.concatenate(cols, axis=1))
        if table_np is None:
            import ml_dtypes
            S_np = S_np.astype(ml_dtypes.bfloat16)
        per_core.append({"S": S_np, "idx": idx_np})
    return per_core, group_sizes, TC


def _build(group_sizes, TC):
    tdt = _f32 if TABLE_DT == "float32" else _bf16

    def mm(ap):
        # fast-fp32 mode for fp32 matmul operands
        if MM_F32R and ap.dtype == _f32:
            return ap.bitcast(_f32r)
        return ap

    nc = bacc.Bacc("TRN2", target_bir_lowering=False, debug=False,
                   num_devices=NCORES)
    xT_d = nc.dram_tensor("xT", [D, RPAD], _f32, kind="ExternalInput")
    dinv_d = nc.dram_tensor("dinv", [RPAD], _f32, kind="ExternalInput")
    W_d = {k: nc.dram_tensor(k, [D, D], _f32, kind="ExternalInput")
           for k in ("W1", "W2", "Wf1")}
    Wf2_d = nc.dram_tensor("Wf2", [D, D_OUT], _f32, kind="ExternalInput")
    bb_d = {k: nc.dram_tensor(k, [128, D], _f32, kind="ExternalInput")
            for k in ("b1", "b2")}
    bf1_d = nc.dram_tensor("bf1", [D], _f32, kind="ExternalInput")
    bf2_d = nc.dram_tensor("bf2", [128, D_OUT], _f32, kind="ExternalInput")
    S_d = nc.dram_tensor("S", [TC, 128, 128], tdt, kind="ExternalInput")
    idx_d = nc.dram_tensor("idx", [128, TC * 8], _i16, kind="ExternalInput")
    out_d = nc.dram_tensor("out", [RPAD, D_OUT], _f32, kind="ExternalOutput")

    cc_in = [nc.dram_tensor(f"cc_in{i}", [RPAD, D], tdt, kind="Internal")
             for i in (1, 2)]
    cc_out = [nc.dram_tensor(f"cc_out{i}", [NPAD, D], tdt, kind="Internal",
                             addr_space="Shared") for i in (1, 2)]

    RG = [list(range(NCORES))]
    ACT = mybir.ActivationFunctionType

    with tile.TileContext(nc) as tc, ExitStack() as ctx:
        const = ctx.enter_context(tc.tile_pool(name="const", bufs=1))
        actT = ctx.enter_context(tc.tile_pool(name="actT", bufs=2))
        work = ctx.enter_context(tc.tile_pool(name="work", bufs=2))
        msgp = ctx.enter_context(tc.tile_pool(name="msgp", bufs=2))
        sp = ctx.enter_context(tc.tile_pool(name="sp", bufs=2))
        psA = ctx.enter_context(tc.tile_pool(name="psA", bufs=2, space="PSUM"))
        psC = ctx.enter_context(tc.tile_pool(name="psC", bufs=2, space="PSUM"))
        psT = ctx.enter_context(tc.tile_pool(name="psT", bufs=2, space="PSUM"))

        # ---- constants ----
        w_t = {}
        for k in ("W1", "W2", "Wf1"):
            w_t[k] = const.tile([128, 4, D], _f32, name=f"wt_{k}")
            nc.sync.dma_start(w_t[k][:], W_d[k].ap().rearrange("(k p) n -> p k n", p=128))
        wf2_t = const.tile([128, 4, D_OUT], _f32)
        nc.sync.dma_start(wf2_t[:], Wf2_d.ap().rearrange("(k p) n -> p k n", p=128))
        b_b = {}
        for k in ("b1", "b2"):
            b_b[k] = const.tile([128, D], _f32, name=f"bb_{k}")
            nc.sync.dma_start(b_b[k][:], bb_d[k].ap())
        bf1_t = const.tile([128, 4], _f32)
        nc.sync.dma_start(bf1_t[:], bf1_d.ap().rearrange("(a p) -> p a", p=128))
        bf2_b = const.tile([128, D_OUT], _f32)
        nc.sync.dma_start(bf2_b[:], bf2_d.ap())
        dinv_t = const.tile([128, MT], _f32)
        nc.sync.dma_start(dinv_t[:], dinv_d.ap().rearrange("(a p) -> p a", p=128))
        ident = const.tile([128, 128], _f32)
        make_identity(nc, ident[:])
        idx_t = const.tile([128, TC * 8], _i16)
        nc.sync.dma_start(idx_t[:], idx_d.ap())

        def phase_a(srcT, wt, cc):
            # cc[m-tile] = dinv * (z @ W) for this core's rows
            for m in range(MT):
                ps = psA.tile([128, D], _f32, tag="psA")
                for k in range(4):
                    nc.tensor.matmul(ps[:], lhsT=mm(srcT[:, k, m * 128:(m + 1) * 128]),
                                     rhs=mm(wt[:, k, :]), start=(k == 0), stop=(k == 3))
                hs = work.tile([128, D], tdt, tag="hs")
                nc.scalar.activation(hs[:], ps[:], ACT.Copy, scale=dinv_t[:, m:m + 1])
                nc.sync.dma_start(cc.ap()[m * 128:(m + 1) * 128, :], hs[:])

        def phase_c(cc, zT_next, bias_b):
            # zT_next = relu(S^T-aggregated messages + b), transposed
            t0 = 0
            col0 = 0
            for w in range(MT):
                nchunks = sum(group_sizes[w])
                ps = psC.tile([128, D], _f32, tag="psC")
                done = 0
                for gsz in group_sizes[w]:
                    nidx = gsz * 128
                    msg = msgp.tile([128, G, D], tdt, tag="msg")
                    nc.gpsimd.dma_gather(msg[:, :gsz, :], cc.ap(),
                                         idx_t[:, col0:col0 + gsz * 8],
                                         nidx, nidx, D)
                    s_t = sp.tile([128, G, 128], tdt, tag="S")
                    nc.sync.dma_start(s_t[:, :gsz, :],
                                      S_d.ap()[t0:t0 + gsz].rearrange("c e j -> e c j"))
                    for t in range(gsz):
                        nc.tensor.matmul(ps[:], lhsT=mm(s_t[:, t, :]), rhs=mm(msg[:, t, :]),
                                         start=(done == 0), stop=(done == nchunks - 1))
                        done += 1
                    t0 += gsz
                    col0 += gsz * 8
                zsum = work.tile([128, D], _f32, tag="zsum")
                nc.vector.tensor_tensor(zsum[:], ps[:], bias_b[:], op=mybir.AluOpType.add)
                zrel = work.tile([128, D], _f32, tag="zrel")
                nc.scalar.activation(zrel[:], zsum[:], ACT.Relu)
                for q in range(4):
                    pt = psT.tile([128, 128], _f32, tag="psT")
                    nc.tensor.transpose(pt[:], zrel[:, q * 128:(q + 1) * 128], ident[:])
                    nc.vector.tensor_copy(zT_next[:, q, w * 128:(w + 1) * 128], pt[:])

        def allgather(i):
            nc.gpsimd.collective_compute(
                "AllGather", mybir.AluOpType.bypass,
                ins=[cc_in[i].ap()], outs=[cc_out[i].ap()], replica_groups=RG)

        # ---- layer 1 ----
        xT_t = actT.tile([128, 4, RPAD], _f32, tag="zT")
        nc.sync.dma_start(xT_t[:], xT_d.ap().rearrange("(k p) m -> p k m", p=128))
        phase_a(xT_t, w_t["W1"], cc_in[0])
        allgather(0)
        z1T = actT.tile([128, 4, RPAD], _f32, tag="zT")
        phase_c(cc_out[0], z1T, b_b["b1"])

        # ---- layer 2 ----
        phase_a(z1T, w_t["W2"], cc_in[1])
        allgather(1)
        z2T = actT.tile([128, 4, RPAD], _f32, tag="zT")
        phase_c(cc_out[1], z2T, b_b["b2"])

        # ---- head: z3 = relu(z2 @ Wf1 + bf1), out = softmax(z3 @ Wf2 + bf2) ----
        z3T = actT.tile([128, 4, RPAD], _f32, tag="zT")
        for q in range(4):
            for mb in range(RPAD // 512):
                ps = psA.tile([128, D], _f32, tag="psA")
                for k in range(4):
                    nc.tensor.matmul(ps[:], lhsT=mm(w_t["Wf1"][:, k, q * 128:(q + 1) * 128]),
                                     rhs=mm(z2T[:, k, mb * 512:(mb + 1) * 512]),
                                     start=(k == 0), stop=(k == 3))
                nc.scalar.activation(z3T[:, q, mb * 512:(mb + 1) * 512], ps[:],
                                     ACT.Relu, bias=bf1_t[:, q:q + 1])
        for m in range(MT):
            ps2 = psT.tile([128, D_OUT], _f32, tag="psT")
            for k in range(4):
                nc.tensor.matmul(ps2[:], lhsT=mm(z3T[:, k, m * 128:(m + 1) * 128]),
                                 rhs=mm(wf2_t[:, k, :]), start=(k == 0), stop=(k == 3))
            lg = work.tile([128, D_OUT], _f32, tag="lg")
            nc.vector.tensor_tensor(lg[:], ps2[:], bf2_b[:], op=mybir.AluOpType.add)
            nmx = work.tile([128, 1], _f32, tag="nmx")
            nc.vector.tensor_reduce(nmx[:], lg[:], axis=mybir.AxisListType.X,
                                    op=mybir.AluOpType.max, negate=True)
            ex = work.tile([128, D_OUT], _f32, tag="ex")
            sm = work.tile([128, 1], _f32, tag="sm")
            nc.scalar.activation(ex[:], lg[:], ACT.Exp, bias=nmx[:, :1], scale=1.0,
                                 accum_out=sm[:, :1])
            rin = work.tile([128, 1], _f32, tag="rin")
            nc.vector.reciprocal(rin[:], sm[:])
            ot = work.tile([128, D_OUT], _f32, tag="ot")
            nc.vector.tensor_scalar_mul(ot[:], ex[:], rin[:, :1])
            nc.sync.dma_start(out_d.ap()[m * 128:(m + 1) * 128, :], ot[:])

    nc.compile()
    return nc


def _run(inputs, trace=False):
    x = np.asarray(inputs["x"], dtype=np.float32)
    edge_index = np.asarray(inputs["edge_index"])
    deg = np.bincount(
        np.concatenate([edge_index[1], np.arange(N, dtype=edge_index.dtype)]),
        minlength=N,
    ).astype(np.float32)
    dinv = np.zeros(N, dtype=np.float32)
    nz = deg > 0
    dinv[nz] = (1.0 / np.sqrt(deg[nz])).astype(np.float32)

    per_core, group_sizes, TC = _prepare(x, edge_index, dinv)
    nc = _build(group_sizes, TC)

    in_maps = []
    for c in range(NCORES):
        xp = np.zeros((RPAD, D), dtype=np.float32)
        xp[:RPC] = x[c * RPC:(c + 1) * RPC]
        dv = np.zeros(RPAD, dtype=np.float32)
        dv[:RPC] = dinv[c * RPC:(c + 1) * RPC]
        bb = {k: np.broadcast_to(np.asarray(inputs[k], np.float32), (128, D)).copy()
              for k in ("b1", "b2")}
        in_maps.append({
            "xT": np.ascontiguousarray(xp.T),
            "dinv": dv,
            "W1": np.asarray(inputs["W1"], np.float32),
            "W2": np.asarray(inputs["W2"], np.float32),
            "Wf1": np.asarray(inputs["Wf1"], np.float32),
            "Wf2": np.asarray(inputs["Wf2"], np.float32),
            "b1": bb["b1"],
            "b2": bb["b2"],
            "bf1": np.asarray(inputs["bf1"], np.float32),
            "bf2": np.broadcast_to(np.asarray(inputs["bf2"], np.float32),
                                   (128, D_OUT)).copy(),
            "S": per_core[c]["S"],
            "idx": per_core[c]["idx"],
        })

    res = run_bass_kernel_spmd(nc, in_maps, core_ids=list(range(NCORES)),
                               trace=trace)
    out = np.concatenate([res.results[c]["out"][:RPC] for c in range(NCORES)], axis=0)
    return out, res


def kernel(**inputs):
    out, _ = _run(inputs, trace=False)
    return out


# revision 4
# speedup vs baseline: 1.6964x; 1.6964x over previous
"""GCN (2x GCNConv + MLP head + softmax) on 8 TRN2 NeuronCores.

Strategy (graph/data parallel, per sharding hint):
  - Nodes are sharded across 8 cores (2500 rows each, padded to 2560).
  - Weight matrices replicated.
  - Per layer: each core computes h = z @ W for its rows, pre-scales by
    dinv (deg^-1/2), AllGathers the scaled table (node-major), then
    aggregates messages for edges partitioned by dst (local windows of
    128 dst nodes) using dma_gather (row gather) + one-hot segment
    matmuls on the TensorEngine accumulating in PSUM. dinv[dst] is
    folded into the segment matrices host-side; self-loops are ordinary
    edges. Epilogue adds bias, applies relu, and PE-transposes into the
    feature-major layout the next matmul needs as lhsT.
  - Head: two dense layers + row softmax, all local.

Host-side preprocessing is limited to graph-structure work (edge sort,
degree counts, one-hot segment matrices, index layout) + sharding.
"""

import os
from contextlib import ExitStack

import numpy as np

import concourse.bacc as bacc
import concourse.mybir as mybir
import concourse.tile as tile
from concourse.bass_utils import run_bass_kernel_spmd
from concourse.masks import make_identity

# problem shapes (hardcoded per contract)
N = 20000
E = 320000
D = 512
D_OUT = 128
NCORES = 8
RPC = 2500          # real rows per core
RPAD = 2560         # padded rows per core (20 tiles of 128)
NPAD = RPAD * NCORES
MT = RPAD // 128    # m-tiles / dst windows per core (20)
G = 6               # max chunks (of 128 edges) per dma_gather call

# config: "f32" (exact), "f32r" (fast fp32 matmul), "bf16" (half-traffic)
MODE = os.environ.get("GNN_MODE", "f32")

_f32 = mybir.dt.float32
_f32r = mybir.dt.float32r
_bf16 = mybir.dt.bfloat16
_i16 = mybir.dt.int16


def _pad_id(r):
    return r + 60 * (r // RPC)


def _prepare(x, edge_index, dinv):
    """Edge partitioning by dst + per-core S matrices and gather indices.

    Returns (per_core: list of dict, group_sizes: list[list[int]], TC).
    """
    src = np.concatenate([edge_index[0], np.arange(N, dtype=np.int64)])
    dst = np.concatenate([edge_index[1], np.arange(N, dtype=np.int64)])

    order = np.argsort(dst, kind="stable")
    srcs = src[order]
    dsts = dst[order]
    srcs_pad = _pad_id(srcs).astype(np.int64)
    dinv_dst = dinv[dsts]

    # per-(core, window) counts
    counts = np.zeros((NCORES, MT), dtype=np.int64)
    core_bounds = np.searchsorted(dsts, np.arange(NCORES + 1) * RPC)
    for c in range(NCORES):
        lo, hi = core_bounds[c], core_bounds[c + 1]
        d = dsts[lo:hi] - c * RPC
        wb = np.searchsorted(d, np.arange(MT + 1) * 128)
        counts[c] = wb[1:] - wb[:-1]

    cpw = np.maximum(1, -(-counts.max(axis=0) // 128))  # chunks per window
    TC = int(cpw.sum())
    chunk_base = np.concatenate([[0], np.cumsum(cpw)])[:-1]

    group_sizes = []
    for w in range(MT):
        n = int(cpw[w])
        gs = [G] * (n // G)
        if n % G:
            gs.append(n % G)
        group_sizes.append(gs)

    per_core = []
    for c in range(NCORES):
        S_np = np.zeros((TC, 128, 128), dtype=np.float32)
        gidx = np.zeros((TC, 128), dtype=np.int16)
        lo, hi = core_bounds[c], core_bounds[c + 1]
        d = dsts[lo:hi] - c * RPC
        s_ids = srcs_pad[lo:hi]
        dv = dinv_dst[lo:hi]
        wb = np.searchsorted(d, np.arange(MT + 1) * 128)
        for w in range(MT):
            a, b = wb[w], wb[w + 1]
            k = np.arange(b - a)
            tg = chunk_base[w] + (k // 128)
            row = k % 128
            S_np[tg, row, d[a:b] - w * 128] = dv[a:b]
            gidx[tg, row] = s_ids[a:b]
        # wrapped int16 index layout per gather call
        cols = []
        for w in range(MT):
            t0 = chunk_base[w]
            for gsz in group_sizes[w]:
                L = gidx[t0:t0 + gsz].reshape(-1)
                blk = L.reshape(-1, 16).T  # [16, nidx/16]
                cols.append(np.tile(blk, (8, 1)))
                t0 += gsz
        idx_np = np.ascontiguousarray(np.concatenate(cols, axis=1))
        if MODE == "bf16":
            import ml_dtypes
            S_np = S_np.astype(ml_dtypes.bfloat16)
        per_core.append({"S": S_np, "idx": idx_np})
    return per_core, group_sizes, TC


def _build(group_sizes, TC):
    # mdt: matmul-operand dtype; tdt: gathered-table dtype; trdt: transpose dtype
    mdt = {"f32": _f32, "f32r": _f32r, "bf16": _bf16}[MODE]
    tdt = _bf16 if MODE == "bf16" else _f32
    trdt = _bf16 if MODE == "bf16" else _f32

    nc = bacc.Bacc("TRN2", target_bir_lowering=False, debug=False,
                   num_devices=NCORES)
    xT_d = nc.dram_tensor("xT", [D, RPAD], mdt, kind="ExternalInput")
    dinv_d = nc.dram_tensor("dinv", [RPAD], _f32, kind="ExternalInput")
    W_d = {k: nc.dram_tensor(k, [D, D], mdt, kind="ExternalInput")
           for k in ("W1", "W2", "Wf1")}
    Wf2_d = nc.dram_tensor("Wf2", [D, D_OUT], mdt, kind="ExternalInput")
    bb_d = {k: nc.dram_tensor(k, [128, D], _f32, kind="ExternalInput")
            for k in ("b1", "b2")}
    bf1_d = nc.dram_tensor("bf1", [D], _f32, kind="ExternalInput")
    bf2_d = nc.dram_tensor("bf2", [128, D_OUT], _f32, kind="ExternalInput")
    S_d = nc.dram_tensor("S", [TC, 128, 128], mdt, kind="ExternalInput")
    idx_d = nc.dram_tensor("idx", [128, TC * 8], _i16, kind="ExternalInput")
    out_d = nc.dram_tensor("out", [RPAD, D_OUT], _f32, kind="ExternalOutput")

    cc_in = [nc.dram_tensor(f"cc_in{i}", [RPAD, D], tdt, kind="Internal")
             for i in (1, 2)]
    cc_out = [nc.dram_tensor(f"cc_out{i}", [NPAD, D], tdt, kind="Internal",
                             addr_space="Shared") for i in (1, 2)]

    RG = [list(range(NCORES))]
    ACT = mybir.ActivationFunctionType

    with tile.TileContext(nc) as tc, ExitStack() as ctx:
        const = ctx.enter_context(tc.tile_pool(name="const", bufs=1))
        actT = ctx.enter_context(tc.tile_pool(name="actT", bufs=2))
        work = ctx.enter_context(tc.tile_pool(name="work", bufs=2))
        msgp = ctx.enter_context(tc.tile_pool(name="msgp", bufs=2))
        sp = ctx.enter_context(tc.tile_pool(name="sp", bufs=2))
        psA = ctx.enter_context(tc.tile_pool(name="psA", bufs=2, space="PSUM"))
        psC = ctx.enter_context(tc.tile_pool(name="psC", bufs=2, space="PSUM"))
        psT = ctx.enter_context(tc.tile_pool(name="psT", bufs=2, space="PSUM"))

        # ---- constants ----
        w_t = {}
        for k in ("W1", "W2", "Wf1"):
            w_t[k] = const.tile([128, 4, D], mdt, name=f"wt_{k}")
            nc.sync.dma_start(w_t[k][:], W_d[k].ap().rearrange("(k p) n -> p k n", p=128))
        wf2_t = const.tile([128, 4, D_OUT], mdt)
        nc.sync.dma_start(wf2_t[:], Wf2_d.ap().rearrange("(k p) n -> p k n", p=128))
        b_b = {}
        for k in ("b1", "b2"):
            b_b[k] = const.tile([128, D], _f32, name=f"bb_{k}")
            nc.sync.dma_start(b_b[k][:], bb_d[k].ap())
        bf1_t = const.tile([128, 4], _f32)
        nc.sync.dma_start(bf1_t[:], bf1_d.ap().rearrange("(a p) -> p a", p=128))
        bf2_b = const.tile([128, D_OUT], _f32)
        nc.sync.dma_start(bf2_b[:], bf2_d.ap())
        dinv_t = const.tile([128, MT], _f32)
        nc.sync.dma_start(dinv_t[:], dinv_d.ap().rearrange("(a p) -> p a", p=128))
        ident = const.tile([128, 128], trdt)
        make_identity(nc, ident[:])
        idx_t = const.tile([128, TC * 8], _i16)
        nc.sync.dma_start(idx_t[:], idx_d.ap())

        def phase_a(srcT, wt, cc):
            # cc[m-tile] = dinv * (z @ W) for this core's rows
            for m in range(MT):
                ps = psA.tile([128, D], _f32, tag="psA")
                for k in range(4):
                    nc.tensor.matmul(ps[:], lhsT=srcT[:, k, m * 128:(m + 1) * 128],
                                     rhs=wt[:, k, :], start=(k == 0), stop=(k == 3))
                hs = work.tile([128, D], tdt, tag="hs")
                nc.scalar.activation(hs[:], ps[:], ACT.Copy, scale=dinv_t[:, m:m + 1])
                nc.sync.dma_start(cc.ap()[m * 128:(m + 1) * 128, :], hs[:])

        def phase_c(cc, zT_next, bias_b):
            # zT_next = relu(S^T-aggregated messages + b), transposed
            t0 = 0
            col0 = 0
            for w in range(MT):
                nchunks = sum(group_sizes[w])
                ps = psC.tile([128, D], _f32, tag="psC")
                done = 0
                for gsz in group_sizes[w]:
                    nidx = gsz * 128
                    msg = msgp.tile([128, G, D], mdt, tag="msg")
                    nc.gpsimd.dma_gather(msg[:, :gsz, :], cc.ap().bitcast(mdt),
                                         idx_t[:, col0:col0 + gsz * 8],
                                         nidx, nidx, D)
                    s_t = sp.tile([128, G, 128], mdt, tag="S")
                    nc.sync.dma_start(s_t[:, :gsz, :],
                                      S_d.ap()[t0:t0 + gsz].rearrange("c e j -> e c j"))
                    for t in range(gsz):
                        nc.tensor.matmul(ps[:], lhsT=s_t[:, t, :], rhs=msg[:, t, :],
                                         start=(done == 0), stop=(done == nchunks - 1))
                        done += 1
                    t0 += gsz
                    col0 += gsz * 8
                zsum = work.tile([128, D], _f32, tag="zsum")
                nc.vector.tensor_tensor(zsum[:], ps[:], bias_b[:], op=mybir.AluOpType.add)
                zrel = work.tile([128, D], trdt, tag="zrel")
                nc.scalar.activation(zrel[:], zsum[:], ACT.Relu)
                for q in range(4):
                    pt = psT.tile([128, 128], trdt, tag="psT")
                    nc.tensor.transpose(pt[:], zrel[:, q * 128:(q + 1) * 128], ident[:])
                    nc.vector.tensor_copy(zT_next[:, q, w * 128:(w + 1) * 128], pt[:])

        def allgather(i):
            nc.gpsimd.collective_compute(
                "AllGather", mybir.AluOpType.bypass,
                ins=[cc_in[i].ap()], outs=[cc_out[i].ap()], replica_groups=RG)

        # ---- layer 1 ----
        xT_t = actT.tile([128, 4, RPAD], mdt, tag="zT")
        nc.sync.dma_start(xT_t[:], xT_d.ap().rearrange("(k p) m -> p k m", p=128))
        phase_a(xT_t, w_t["W1"], cc_in[0])
        allgather(0)
        z1T = actT.tile([128, 4, RPAD], mdt, tag="zT")
        phase_c(cc_out[0], z1T, b_b["b1"])

        # ---- layer 2 ----
        phase_a(z1T, w_t["W2"], cc_in[1])
        allgather(1)
        z2T = actT.tile([128, 4, RPAD], mdt, tag="zT")
        phase_c(cc_out[1], z2T, b_b["b2"])

        # ---- head: z3 = relu(z2 @ Wf1 + bf1), out = softmax(z3 @ Wf2 + bf2) ----
        z3T = actT.tile([128, 4, RPAD], mdt, tag="zT")
        for q in range(4):
            for mb in range(RPAD // 512):
                ps = psA.tile([128, D], _f32, tag="psA")
                for k in range(4):
                    nc.tensor.matmul(ps[:], lhsT=w_t["Wf1"][:, k, q * 128:(q + 1) * 128],
                                     rhs=z2T[:, k, mb * 512:(mb + 1) * 512],
                                     start=(k == 0), stop=(k == 3))
                nc.scalar.activation(z3T[:, q, mb * 512:(mb + 1) * 512], ps[:],
                                     ACT.Relu, bias=bf1_t[:, q:q + 1])
        for m in range(MT):
            ps2 = psT.tile([128, D_OUT], _f32, tag="psT")
            for k in range(4):
                nc.tensor.matmul(ps2[:], lhsT=z3T[:, k, m * 128:(m + 1) * 128],
                                 rhs=wf2_t[:, k, :], start=(k == 0), stop=(k == 3))
            lg = work.tile([128, D_OUT], _f32, tag="lg")
            nc.vector.tensor_tensor(lg[:], ps2[:], bf2_b[:], op=mybir.AluOpType.add)
            nmx = work.tile([128, 1], _f32, tag="nmx")
            nc.vector.tensor_reduce(nmx[:], lg[:], axis=mybir.AxisListType.X,
                                    op=mybir.AluOpType.max, negate=True)
            ex = work.tile([128, D_OUT], _f32, tag="ex")
            sm = work.tile([128, 1], _f32, tag="sm")
            nc.scalar.activation(ex[:], lg[:], ACT.Exp, bias=nmx[:, :1], scale=1.0,
                                 accum_out=sm[:, :1])
            rin = work.tile([128, 1], _f32, tag="rin")
            nc.vector.reciprocal(rin[:], sm[:])
            ot = work.tile([128, D_OUT], _f32, tag="ot")
            nc.vector.tensor_scalar_mul(ot[:], ex[:], rin[:, :1])
            nc.sync.dma_start(out_d.ap()[m * 128:(m + 1) * 128, :], ot[:])

    nc.compile()
    return nc


def _run(inputs, trace=False):
    x = np.asarray(inputs["x"], dtype=np.float32)
    edge_index = np.asarray(inputs["edge_index"])
    deg = np.bincount(
        np.concatenate([edge_index[1], np.arange(N, dtype=edge_index.dtype)]),
        minlength=N,
    ).astype(np.float32)
    dinv = np.zeros(N, dtype=np.float32)
    nz = deg > 0
    dinv[nz] = (1.0 / np.sqrt(deg[nz])).astype(np.float32)

    per_core, group_sizes, TC = _prepare(x, edge_index, dinv)
    nc = _build(group_sizes, TC)

    if MODE == "bf16":
        import ml_dtypes
        mnp = ml_dtypes.bfloat16
    else:
        mnp = np.float32

    in_maps = []
    for c in range(NCORES):
        xp = np.zeros((RPAD, D), dtype=np.float32)
        xp[:RPC] = x[c * RPC:(c + 1) * RPC]
        dv = np.zeros(RPAD, dtype=np.float32)
        dv[:RPC] = dinv[c * RPC:(c + 1) * RPC]
        bb = {k: np.broadcast_to(np.asarray(inputs[k], np.float32), (128, D)).copy()
              for k in ("b1", "b2")}
        in_maps.append({
            "xT": np.ascontiguousarray(xp.T).astype(mnp),
            "dinv": dv,
            "W1": np.asarray(inputs["W1"], np.float32).astype(mnp),
            "W2": np.asarray(inputs["W2"], np.float32).astype(mnp),
            "Wf1": np.asarray(inputs["Wf1"], np.float32).astype(mnp),
            "Wf2": np.asarray(inputs["Wf2"], np.float32).astype(mnp),
            "b1": bb["b1"],
            "b2": bb["b2"],
            "bf1": np.asarray(inputs["bf1"], np.float32),
            "bf2": np.broadcast_to(np.asarray(inputs["bf2"], np.float32),
                                   (128, D_OUT)).copy(),
            "S": per_core[c]["S"],
            "idx": per_core[c]["idx"],
        })

    res = run_bass_kernel_spmd(nc, in_maps, core_ids=list(range(NCORES)),
                               trace=trace)
    out = np.concatenate([res.results[c]["out"][:RPC] for c in range(NCORES)], axis=0)
    return out, res


def kernel(**inputs):
    out, _ = _run(inputs, trace=False)
    return out


# revision 5
# speedup vs baseline: 1.7425x; 1.0272x over previous
"""GCN (2x GCNConv + MLP head + softmax) on 8 TRN2 NeuronCores.

Strategy (graph/data parallel, per sharding hint):
  - Nodes are sharded across 8 cores (2500 rows each, padded to 2560).
  - Weight matrices replicated.
  - Per layer: each core computes h = z @ W for its rows, pre-scales by
    dinv (deg^-1/2), AllGathers the scaled table (node-major), then
    aggregates messages for edges partitioned by dst (local windows of
    128 dst nodes) using dma_gather (row gather) + one-hot segment
    matmuls on the TensorEngine accumulating in PSUM. dinv[dst] is
    folded into the segment matrices host-side; self-loops are ordinary
    edges. Epilogue adds bias, applies relu, and PE-transposes into the
    feature-major layout the next matmul needs as lhsT.
  - Head: two dense layers + row softmax, all local.

Host-side preprocessing is limited to graph-structure work (edge sort,
degree counts, one-hot segment matrices, index layout) + sharding.
"""

import os
from contextlib import ExitStack

import numpy as np

import concourse.bacc as bacc
import concourse.mybir as mybir
import concourse.tile as tile
from concourse.bass_utils import run_bass_kernel_spmd
from concourse.masks import make_identity

# problem shapes (hardcoded per contract)
N = 20000
E = 320000
D = 512
D_OUT = 128
NCORES = 8
RPC = 2500          # real rows per core
RPAD = 2560         # padded rows per core (20 tiles of 128)
NPAD = RPAD * NCORES
MT = RPAD // 128    # m-tiles / dst windows per core (20)
G = 6               # max chunks (of 128 edges) per dma_gather call

# config: "f32" (exact), "f32r" (fast fp32 matmul), "bf16" (half-traffic)
MODE = os.environ.get("GNN_MODE", "f32")

_f32 = mybir.dt.float32
_f32r = mybir.dt.float32r
_bf16 = mybir.dt.bfloat16
_i16 = mybir.dt.int16


def _pad_id(r):
    return r + 60 * (r // RPC)


def _prepare(x, edge_index, dinv):
    """Edge partitioning by dst + per-core S matrices and gather indices.

    Returns (per_core: list of dict, group_sizes: list[list[int]], TC).
    """
    src = np.concatenate([edge_index[0], np.arange(N, dtype=np.int64)])
    dst = np.concatenate([edge_index[1], np.arange(N, dtype=np.int64)])

    order = np.argsort(dst, kind="stable")
    srcs = src[order]
    dsts = dst[order]
    srcs_pad = _pad_id(srcs).astype(np.int64)
    dinv_dst = dinv[dsts]

    # per-(core, window) counts
    counts = np.zeros((NCORES, MT), dtype=np.int64)
    core_bounds = np.searchsorted(dsts, np.arange(NCORES + 1) * RPC)
    for c in range(NCORES):
        lo, hi = core_bounds[c], core_bounds[c + 1]
        d = dsts[lo:hi] - c * RPC
        wb = np.searchsorted(d, np.arange(MT + 1) * 128)
        counts[c] = wb[1:] - wb[:-1]

    cpw = np.maximum(1, -(-counts.max(axis=0) // 128))  # chunks per window
    TC = int(cpw.sum())
    chunk_base = np.concatenate([[0], np.cumsum(cpw)])[:-1]

    group_sizes = []
    for w in range(MT):
        n = int(cpw[w])
        gs = [G] * (n // G)
        if n % G:
            gs.append(n % G)
        group_sizes.append(gs)

    per_core = []
    for c in range(NCORES):
        S_np = np.zeros((TC, 128, 128), dtype=np.float32)
        gidx = np.zeros((TC, 128), dtype=np.int16)
        lo, hi = core_bounds[c], core_bounds[c + 1]
        d = dsts[lo:hi] - c * RPC
        s_ids = srcs_pad[lo:hi]
        dv = dinv_dst[lo:hi]
        wb = np.searchsorted(d, np.arange(MT + 1) * 128)
        for w in range(MT):
            a, b = wb[w], wb[w + 1]
            k = np.arange(b - a)
            tg = chunk_base[w] + (k // 128)
            row = k % 128
            S_np[tg, row, d[a:b] - w * 128] = dv[a:b]
            gidx[tg, row] = s_ids[a:b]
        # wrapped int16 index layout per gather call
        cols = []
        for w in range(MT):
            t0 = chunk_base[w]
            for gsz in group_sizes[w]:
                L = gidx[t0:t0 + gsz].reshape(-1)
                blk = L.reshape(-1, 16).T  # [16, nidx/16]
                cols.append(np.tile(blk, (8, 1)))
                t0 += gsz
        idx_np = np.ascontiguousarray(np.concatenate(cols, axis=1))
        if MODE == "bf16":
            import ml_dtypes
            S_np = S_np.astype(ml_dtypes.bfloat16)
        per_core.append({"S": S_np, "idx": idx_np})
    return per_core, group_sizes, TC


def _build(group_sizes, TC):
    # mdt: matmul-operand dtype; tdt: gathered-table dtype; trdt: transpose dtype
    mdt = {"f32": _f32, "f32r": _f32r, "bf16": _bf16}[MODE]
    tdt = _bf16 if MODE == "bf16" else _f32
    trdt = _bf16 if MODE == "bf16" else _f32

    nc = bacc.Bacc("TRN2", target_bir_lowering=False, debug=False,
                   num_devices=NCORES, num_swdge_queues=4)
    xT_d = nc.dram_tensor("xT", [D, RPAD], mdt, kind="ExternalInput")
    dinv_d = nc.dram_tensor("dinv", [RPAD], _f32, kind="ExternalInput")
    W_d = {k: nc.dram_tensor(k, [D, D], mdt, kind="ExternalInput")
           for k in ("W1", "W2", "Wf1")}
    Wf2_d = nc.dram_tensor("Wf2", [D, D_OUT], mdt, kind="ExternalInput")
    bb_d = {k: nc.dram_tensor(k, [128, D], _f32, kind="ExternalInput")
            for k in ("b1", "b2")}
    bf1_d = nc.dram_tensor("bf1", [D], _f32, kind="ExternalInput")
    bf2_d = nc.dram_tensor("bf2", [128, D_OUT], _f32, kind="ExternalInput")
    S_d = nc.dram_tensor("S", [TC, 128, 128], mdt, kind="ExternalInput")
    idx_d = nc.dram_tensor("idx", [128, TC * 8], _i16, kind="ExternalInput")
    out_d = nc.dram_tensor("out", [RPAD, D_OUT], _f32, kind="ExternalOutput")

    cc_in = [nc.dram_tensor(f"cc_in{i}", [RPAD, D], tdt, kind="Internal")
             for i in (1, 2)]
    cc_out = [nc.dram_tensor(f"cc_out{i}", [NPAD, D], tdt, kind="Internal",
                             addr_space="Shared") for i in (1, 2)]

    RG = [list(range(NCORES))]
    ACT = mybir.ActivationFunctionType

    with tile.TileContext(nc) as tc, ExitStack() as ctx:
        const = ctx.enter_context(tc.tile_pool(name="const", bufs=1))
        actT = ctx.enter_context(tc.tile_pool(name="actT", bufs=2))
        work = ctx.enter_context(tc.tile_pool(name="work", bufs=2))
        msgp = ctx.enter_context(tc.tile_pool(name="msgp", bufs=2))
        sp = ctx.enter_context(tc.tile_pool(name="sp", bufs=2))
        psA = ctx.enter_context(tc.tile_pool(name="psA", bufs=2, space="PSUM"))
        psC = ctx.enter_context(tc.tile_pool(name="psC", bufs=2, space="PSUM"))
        psT = ctx.enter_context(tc.tile_pool(name="psT", bufs=2, space="PSUM"))

        # ---- constants ----
        w_t = {}
        for k in ("W1", "W2", "Wf1"):
            w_t[k] = const.tile([128, 4, D], mdt, name=f"wt_{k}")
            nc.sync.dma_start(w_t[k][:], W_d[k].ap().rearrange("(k p) n -> p k n", p=128))
        wf2_t = const.tile([128, 4, D_OUT], mdt)
        nc.sync.dma_start(wf2_t[:], Wf2_d.ap().rearrange("(k p) n -> p k n", p=128))
        b_b = {}
        for k in ("b1", "b2"):
            b_b[k] = const.tile([128, D], _f32, name=f"bb_{k}")
            nc.sync.dma_start(b_b[k][:], bb_d[k].ap())
        bf1_t = const.tile([128, 4], _f32)
        nc.sync.dma_start(bf1_t[:], bf1_d.ap().rearrange("(a p) -> p a", p=128))
        bf2_b = const.tile([128, D_OUT], _f32)
        nc.sync.dma_start(bf2_b[:], bf2_d.ap())
        dinv_t = const.tile([128, MT], _f32)
        nc.sync.dma_start(dinv_t[:], dinv_d.ap().rearrange("(a p) -> p a", p=128))
        ident = const.tile([128, 128], trdt)
        make_identity(nc, ident[:])
        idx_t = const.tile([128, TC * 8], _i16)
        nc.sync.dma_start(idx_t[:], idx_d.ap())

        def phase_a(srcT, wt, cc):
            # cc[m-tile] = dinv * (z @ W) for this core's rows
            for m in range(MT):
                ps = psA.tile([128, D], _f32, tag="psA")
                for k in range(4):
                    nc.tensor.matmul(ps[:], lhsT=srcT[:, k, m * 128:(m + 1) * 128],
                                     rhs=wt[:, k, :], start=(k == 0), stop=(k == 3))
                hs = work.tile([128, D], tdt, tag="hs")
                nc.scalar.activation(hs[:], ps[:], ACT.Copy, scale=dinv_t[:, m:m + 1])
                nc.sync.dma_start(cc.ap()[m * 128:(m + 1) * 128, :], hs[:])

        def phase_c(cc, zT_next, bias_b):
            # zT_next = relu(S^T-aggregated messages + b), transposed
            t0 = 0
            col0 = 0
            qn = [0]
            for w in range(MT):
                nchunks = sum(group_sizes[w])
                ps = psC.tile([128, D], _f32, tag="psC")
                done = 0
                for gsz in group_sizes[w]:
                    nidx = gsz * 128
                    msg = msgp.tile([128, G, D], mdt, tag="msg")
                    nc.gpsimd.dma_gather(msg[:, :gsz, :], cc.ap().bitcast(mdt),
                                         idx_t[:, col0:col0 + gsz * 8],
                                         nidx, nidx, D, queue_num=qn[0] % 4)
                    qn[0] += 1
                    s_t = sp.tile([128, G, 128], mdt, tag="S")
                    nc.sync.dma_start(s_t[:, :gsz, :],
                                      S_d.ap()[t0:t0 + gsz].rearrange("c e j -> e c j"))
                    for t in range(gsz):
                        nc.tensor.matmul(ps[:], lhsT=s_t[:, t, :], rhs=msg[:, t, :],
                                         start=(done == 0), stop=(done == nchunks - 1))
                        done += 1
                    t0 += gsz
                    col0 += gsz * 8
                zsum = work.tile([128, D], _f32, tag="zsum")
                nc.vector.tensor_tensor(zsum[:], ps[:], bias_b[:], op=mybir.AluOpType.add)
                zrel = work.tile([128, D], trdt, tag="zrel")
                nc.scalar.activation(zrel[:], zsum[:], ACT.Relu)
                for q in range(4):
                    pt = psT.tile([128, 128], trdt, tag="psT")
                    nc.tensor.transpose(pt[:], zrel[:, q * 128:(q + 1) * 128], ident[:])
                    nc.vector.tensor_copy(zT_next[:, q, w * 128:(w + 1) * 128], pt[:])

        def allgather(i):
            nc.gpsimd.collective_compute(
                "AllGather", mybir.AluOpType.bypass,
                ins=[cc_in[i].ap()], outs=[cc_out[i].ap()], replica_groups=RG)

        # ---- layer 1 ----
        xT_t = actT.tile([128, 4, RPAD], mdt, tag="zT")
        nc.sync.dma_start(xT_t[:], xT_d.ap().rearrange("(k p) m -> p k m", p=128))
        phase_a(xT_t, w_t["W1"], cc_in[0])
        allgather(0)
        z1T = actT.tile([128, 4, RPAD], mdt, tag="zT")
        phase_c(cc_out[0], z1T, b_b["b1"])

        # ---- layer 2 ----
        phase_a(z1T, w_t["W2"], cc_in[1])
        allgather(1)
        z2T = actT.tile([128, 4, RPAD], mdt, tag="zT")
        phase_c(cc_out[1], z2T, b_b["b2"])

        # ---- head: z3 = relu(z2 @ Wf1 + bf1), out = softmax(z3 @ Wf2 + bf2) ----
        z3T = actT.tile([128, 4, RPAD], mdt, tag="zT")
        for q in range(4):
            for mb in range(RPAD // 512):
                ps = psA.tile([128, D], _f32, tag="psA")
                for k in range(4):
                    nc.tensor.matmul(ps[:], lhsT=w_t["Wf1"][:, k, q * 128:(q + 1) * 128],
                                     rhs=z2T[:, k, mb * 512:(mb + 1) * 512],
                                     start=(k == 0), stop=(k == 3))
                nc.scalar.activation(z3T[:, q, mb * 512:(mb + 1) * 512], ps[:],
                                     ACT.Relu, bias=bf1_t[:, q:q + 1])
        for m in range(MT):
            ps2 = psT.tile([128, D_OUT], _f32, tag="psT")
            for k in range(4):
                nc.tensor.matmul(ps2[:], lhsT=z3T[:, k, m * 128:(m + 1) * 128],
                                 rhs=wf2_t[:, k, :], start=(k == 0), stop=(k == 3))
            lg = work.tile([128, D_OUT], _f32, tag="lg")
            nc.vector.tensor_tensor(lg[:], ps2[:], bf2_b[:], op=mybir.AluOpType.add)
            nmx = work.tile([128, 1], _f32, tag="nmx")
            nc.vector.tensor_reduce(nmx[:], lg[:], axis=mybir.AxisListType.X,
                                    op=mybir.AluOpType.max, negate=True)
            ex = work.tile([128, D_OUT], _f32, tag="ex")
            sm = work.tile([128, 1], _f32, tag="sm")
            nc.scalar.activation(ex[:], lg[:], ACT.Exp, bias=nmx[:, :1], scale=1.0,
                                 accum_out=sm[:, :1])
            rin = work.tile([128, 1], _f32, tag="rin")
            nc.vector.reciprocal(rin[:], sm[:])
            ot = work.tile([128, D_OUT], _f32, tag="ot")
            nc.vector.tensor_scalar_mul(ot[:], ex[:], rin[:, :1])
            nc.sync.dma_start(out_d.ap()[m * 128:(m + 1) * 128, :], ot[:])

    nc.compile()
    return nc


def _run(inputs, trace=False):
    x = np.asarray(inputs["x"], dtype=np.float32)
    edge_index = np.asarray(inputs["edge_index"])
    deg = np.bincount(
        np.concatenate([edge_index[1], np.arange(N, dtype=edge_index.dtype)]),
        minlength=N,
    ).astype(np.float32)
    dinv = np.zeros(N, dtype=np.float32)
    nz = deg > 0
    dinv[nz] = (1.0 / np.sqrt(deg[nz])).astype(np.float32)

    per_core, group_sizes, TC = _prepare(x, edge_index, dinv)
    nc = _build(group_sizes, TC)

    if MODE == "bf16":
        import ml_dtypes
        mnp = ml_dtypes.bfloat16
    else:
        mnp = np.float32

    in_maps = []
    for c in range(NCORES):
        xp = np.zeros((RPAD, D), dtype=np.float32)
        xp[:RPC] = x[c * RPC:(c + 1) * RPC]
        dv = np.zeros(RPAD, dtype=np.float32)
        dv[:RPC] = dinv[c * RPC:(c + 1) * RPC]
        bb = {k: np.broadcast_to(np.asarray(inputs[k], np.float32), (128, D)).copy()
              for k in ("b1", "b2")}
        in_maps.append({
            "xT": np.ascontiguousarray(xp.T).astype(mnp),
            "dinv": dv,
            "W1": np.asarray(inputs["W1"], np.float32).astype(mnp),
            "W2": np.asarray(inputs["W2"], np.float32).astype(mnp),
            "Wf1": np.asarray(inputs["Wf1"], np.float32).astype(mnp),
            "Wf2": np.asarray(inputs["Wf2"], np.float32).astype(mnp),
            "b1": bb["b1"],
            "b2": bb["b2"],
            "bf1": np.asarray(inputs["bf1"], np.float32),
            "bf2": np.broadcast_to(np.asarray(inputs["bf2"], np.float32),
                                   (128, D_OUT)).copy(),
            "S": per_core[c]["S"],
            "idx": per_core[c]["idx"],
        })

    res = run_bass_kernel_spmd(nc, in_maps, core_ids=list(range(NCORES)),
                               trace=trace)
    out = np.concatenate([res.results[c]["out"][:RPC] for c in range(NCORES)], axis=0)
    return out, res


def kernel(**inputs):
    out, _ = _run(inputs, trace=False)
    return out


# revision 6
# speedup vs baseline: 2.3859x; 1.3692x over previous
"""GCN (2x GCNConv + MLP head + softmax) on 8 TRN2 NeuronCores.

Strategy (graph/data parallel, per sharding hint):
  - Nodes are sharded across 8 cores (2500 rows each, padded to 2560).
  - Weight matrices replicated.
  - Per layer: each core computes h = z @ W for its rows, pre-scales by
    dinv (deg^-1/2), AllGathers the scaled table (node-major), then
    aggregates messages for edges partitioned by dst (local windows of
    128 dst nodes) using dma_gather (row gather) + one-hot segment
    matmuls on the TensorEngine accumulating in PSUM. dinv[dst] is
    folded into the segment matrices host-side; self-loops are ordinary
    edges. Epilogue adds bias, applies relu, and PE-transposes into the
    feature-major layout the next matmul needs as lhsT.
  - Head: two dense layers + row softmax, all local.

Host-side preprocessing is limited to graph-structure work (edge sort,
degree counts, one-hot segment matrices, index layout) + sharding.
"""

import os
from contextlib import ExitStack

import numpy as np

import concourse.bacc as bacc
import concourse.mybir as mybir
import concourse.tile as tile
from concourse.bass_utils import run_bass_kernel_spmd
from concourse.masks import make_identity

# problem shapes (hardcoded per contract)
N = 20000
E = 320000
D = 512
D_OUT = 128
NCORES = 8
RPC = 2500          # real rows per core
RPAD = 2560         # padded rows per core (20 tiles of 128)
NPAD = RPAD * NCORES
MT = RPAD // 128    # m-tiles / dst windows per core (20)
G = 6               # max chunks (of 128 edges) per dma_gather call

# config: "f32" (exact), "f32r" (fast fp32 matmul), "bf16" (half-traffic)
MODE = os.environ.get("GNN_MODE", "f32")

_f32 = mybir.dt.float32
_f32r = mybir.dt.float32r
_bf16 = mybir.dt.bfloat16
_i16 = mybir.dt.int16


def _pad_id(r):
    return r + 60 * (r // RPC)


def _prepare(x, edge_index, dinv):
    """Edge partitioning by dst + per-core S matrices and gather indices.

    Returns (per_core: list of dict, group_sizes: list[list[int]], TC).
    """
    src = np.concatenate([edge_index[0], np.arange(N, dtype=np.int64)])
    dst = np.concatenate([edge_index[1], np.arange(N, dtype=np.int64)])

    order = np.argsort(dst, kind="stable")
    srcs = src[order]
    dsts = dst[order]
    srcs_pad = _pad_id(srcs).astype(np.int64)
    dinv_dst = dinv[dsts]

    # per-(core, window) counts
    counts = np.zeros((NCORES, MT), dtype=np.int64)
    core_bounds = np.searchsorted(dsts, np.arange(NCORES + 1) * RPC)
    for c in range(NCORES):
        lo, hi = core_bounds[c], core_bounds[c + 1]
        d = dsts[lo:hi] - c * RPC
        wb = np.searchsorted(d, np.arange(MT + 1) * 128)
        counts[c] = wb[1:] - wb[:-1]

    cpw = np.maximum(1, -(-counts.max(axis=0) // 128))  # chunks per window
    TC = int(cpw.sum())
    chunk_base = np.concatenate([[0], np.cumsum(cpw)])[:-1]

    group_sizes = []
    for w in range(MT):
        n = int(cpw[w])
        gs = [G] * (n // G)
        if n % G:
            gs.append(n % G)
        group_sizes.append(gs)

    per_core = []
    for c in range(NCORES):
        S_np = np.zeros((TC, 128, 128), dtype=np.float32)
        gidx = np.zeros((TC, 128), dtype=np.int16)
        lo, hi = core_bounds[c], core_bounds[c + 1]
        d = dsts[lo:hi] - c * RPC
        s_ids = srcs_pad[lo:hi]
        dv = dinv_dst[lo:hi]
        wb = np.searchsorted(d, np.arange(MT + 1) * 128)
        for w in range(MT):
            a, b = wb[w], wb[w + 1]
            k = np.arange(b - a)
            tg = chunk_base[w] + (k // 128)
            row = k % 128
            S_np[tg, row, d[a:b] - w * 128] = dv[a:b]
            gidx[tg, row] = s_ids[a:b]
        # wrapped int16 index layout per gather call
        cols = []
        for w in range(MT):
            t0 = chunk_base[w]
            for gsz in group_sizes[w]:
                L = gidx[t0:t0 + gsz].reshape(-1)
                blk = L.reshape(-1, 16).T  # [16, nidx/16]
                cols.append(np.tile(blk, (8, 1)))
                t0 += gsz
        idx_np = np.ascontiguousarray(np.concatenate(cols, axis=1))
        if MODE == "bf16":
            import ml_dtypes
            S_np = S_np.astype(ml_dtypes.bfloat16)
        per_core.append({"S": S_np, "idx": idx_np})
    return per_core, group_sizes, TC


def _build(group_sizes, TC):
    # mdt: matmul-operand dtype; tdt: gathered-table dtype; trdt: transpose dtype
    mdt = {"f32": _f32, "f32r": _f32r, "bf16": _bf16}[MODE]
    tdt = _bf16 if MODE == "bf16" else _f32
    trdt = _bf16 if MODE == "bf16" else _f32

    nc = bacc.Bacc("TRN2", target_bir_lowering=False, debug=False,
                   num_devices=NCORES, num_swdge_queues=4)
    xT_d = nc.dram_tensor("xT", [D, RPAD], mdt, kind="ExternalInput")
    dinv_d = nc.dram_tensor("dinv", [RPAD], _f32, kind="ExternalInput")
    W_d = {k: nc.dram_tensor(k, [D, D], mdt, kind="ExternalInput")
           for k in ("W1", "W2", "Wf1")}
    Wf2_d = nc.dram_tensor("Wf2", [D, D_OUT], mdt, kind="ExternalInput")
    bb_d = {k: nc.dram_tensor(k, [128, D], _f32, kind="ExternalInput")
            for k in ("b1", "b2")}
    bf1_d = nc.dram_tensor("bf1", [D], _f32, kind="ExternalInput")
    bf2_d = nc.dram_tensor("bf2", [128, D_OUT], _f32, kind="ExternalInput")
    S_d = nc.dram_tensor("S", [TC, 128, 128], mdt, kind="ExternalInput")
    idx_d = nc.dram_tensor("idx", [128, TC * 8], _i16, kind="ExternalInput")
    out_d = nc.dram_tensor("out", [RPAD, D_OUT], _f32, kind="ExternalOutput")

    cc_in = [nc.dram_tensor(f"cc_in{i}", [RPAD, D], tdt, kind="Internal")
             for i in (1, 2)]
    cc_out = [nc.dram_tensor(f"cc_out{i}", [NPAD, D], tdt, kind="Internal",
                             addr_space="Shared") for i in (1, 2)]

    RG = [list(range(NCORES))]
    ACT = mybir.ActivationFunctionType

    with tile.TileContext(nc) as tc, ExitStack() as ctx:
        const = ctx.enter_context(tc.tile_pool(name="const", bufs=1))
        actT = ctx.enter_context(tc.tile_pool(name="actT", bufs=2))
        work = ctx.enter_context(tc.tile_pool(name="work", bufs=2))
        msgp = ctx.enter_context(tc.tile_pool(name="msgp", bufs=5))
        sp = ctx.enter_context(tc.tile_pool(name="sp", bufs=5))
        psA = ctx.enter_context(tc.tile_pool(name="psA", bufs=2, space="PSUM"))
        psC = ctx.enter_context(tc.tile_pool(name="psC", bufs=2, space="PSUM"))
        psT = ctx.enter_context(tc.tile_pool(name="psT", bufs=2, space="PSUM"))

        # ---- constants ----
        w_t = {}
        for k in ("W1", "W2", "Wf1"):
            w_t[k] = const.tile([128, 4, D], mdt, name=f"wt_{k}")
            nc.sync.dma_start(w_t[k][:], W_d[k].ap().rearrange("(k p) n -> p k n", p=128))
        wf2_t = const.tile([128, 4, D_OUT], mdt)
        nc.sync.dma_start(wf2_t[:], Wf2_d.ap().rearrange("(k p) n -> p k n", p=128))
        b_b = {}
        for k in ("b1", "b2"):
            b_b[k] = const.tile([128, D], _f32, name=f"bb_{k}")
            nc.sync.dma_start(b_b[k][:], bb_d[k].ap())
        bf1_t = const.tile([128, 4], _f32)
        nc.sync.dma_start(bf1_t[:], bf1_d.ap().rearrange("(a p) -> p a", p=128))
        bf2_b = const.tile([128, D_OUT], _f32)
        nc.sync.dma_start(bf2_b[:], bf2_d.ap())
        dinv_t = const.tile([128, MT], _f32)
        nc.sync.dma_start(dinv_t[:], dinv_d.ap().rearrange("(a p) -> p a", p=128))
        ident = const.tile([128, 128], trdt)
        make_identity(nc, ident[:])
        idx_t = const.tile([128, TC * 8], _i16)
        nc.sync.dma_start(idx_t[:], idx_d.ap())

        def phase_a(srcT, wt, cc):
            # cc[m-tile] = dinv * (z @ W) for this core's rows
            for m in range(MT):
                ps = psA.tile([128, D], _f32, tag="psA")
                for k in range(4):
                    nc.tensor.matmul(ps[:], lhsT=srcT[:, k, m * 128:(m + 1) * 128],
                                     rhs=wt[:, k, :], start=(k == 0), stop=(k == 3))
                hs = work.tile([128, D], tdt, tag="hs")
                nc.scalar.activation(hs[:], ps[:], ACT.Copy, scale=dinv_t[:, m:m + 1])
                nc.sync.dma_start(cc.ap()[m * 128:(m + 1) * 128, :], hs[:])

        def phase_c(cc, zT_next, bias_b):
            # zT_next = relu(S^T-aggregated messages + b), transposed
            t0 = 0
            col0 = 0
            qn = [0]
            for w in range(MT):
                nchunks = sum(group_sizes[w])
                ps = psC.tile([128, D], _f32, tag="psC")
                done = 0
                for gsz in group_sizes[w]:
                    nidx = gsz * 128
                    msg = msgp.tile([128, G, D], mdt, tag="msg")
                    nc.gpsimd.dma_gather(msg[:, :gsz, :], cc.ap().bitcast(mdt),
                                         idx_t[:, col0:col0 + gsz * 8],
                                         nidx, nidx, D, queue_num=qn[0] % 4)
                    qn[0] += 1
                    s_t = sp.tile([128, G, 128], mdt, tag="S")
                    nc.sync.dma_start(s_t[:, :gsz, :],
                                      S_d.ap()[t0:t0 + gsz].rearrange("c e j -> e c j"))
                    for t in range(gsz):
                        nc.tensor.matmul(ps[:], lhsT=s_t[:, t, :], rhs=msg[:, t, :],
                                         start=(done == 0), stop=(done == nchunks - 1))
                        done += 1
                    t0 += gsz
                    col0 += gsz * 8
                zsum = work.tile([128, D], _f32, tag="zsum")
                nc.vector.tensor_tensor(zsum[:], ps[:], bias_b[:], op=mybir.AluOpType.add)
                zrel = work.tile([128, D], trdt, tag="zrel")
                nc.scalar.activation(zrel[:], zsum[:], ACT.Relu)
                for q in range(4):
                    pt = psT.tile([128, 128], trdt, tag="psT")
                    nc.tensor.transpose(pt[:], zrel[:, q * 128:(q + 1) * 128], ident[:])
                    nc.vector.tensor_copy(zT_next[:, q, w * 128:(w + 1) * 128], pt[:])

        def allgather(i):
            nc.gpsimd.collective_compute(
                "AllGather", mybir.AluOpType.bypass,
                ins=[cc_in[i].ap()], outs=[cc_out[i].ap()], replica_groups=RG)

        # ---- layer 1 ----
        xT_t = actT.tile([128, 4, RPAD], mdt, tag="zT")
        nc.sync.dma_start(xT_t[:], xT_d.ap().rearrange("(k p) m -> p k m", p=128))
        phase_a(xT_t, w_t["W1"], cc_in[0])
        allgather(0)
        z1T = actT.tile([128, 4, RPAD], mdt, tag="zT")
        phase_c(cc_out[0], z1T, b_b["b1"])

        # ---- layer 2 ----
        phase_a(z1T, w_t["W2"], cc_in[1])
        allgather(1)
        z2T = actT.tile([128, 4, RPAD], mdt, tag="zT")
        phase_c(cc_out[1], z2T, b_b["b2"])

        # ---- head: z3 = relu(z2 @ Wf1 + bf1), out = softmax(z3 @ Wf2 + bf2) ----
        z3T = actT.tile([128, 4, RPAD], mdt, tag="zT")
        for q in range(4):
            for mb in range(RPAD // 512):
                ps = psA.tile([128, D], _f32, tag="psA")
                for k in range(4):
                    nc.tensor.matmul(ps[:], lhsT=w_t["Wf1"][:, k, q * 128:(q + 1) * 128],
                                     rhs=z2T[:, k, mb * 512:(mb + 1) * 512],
                                     start=(k == 0), stop=(k == 3))
                nc.scalar.activation(z3T[:, q, mb * 512:(mb + 1) * 512], ps[:],
                                     ACT.Relu, bias=bf1_t[:, q:q + 1])
        for m in range(MT):
            ps2 = psT.tile([128, D_OUT], _f32, tag="psT")
            for k in range(4):
                nc.tensor.matmul(ps2[:], lhsT=z3T[:, k, m * 128:(m + 1) * 128],
                                 rhs=wf2_t[:, k, :], start=(k == 0), stop=(k == 3))
            lg = work.tile([128, D_OUT], _f32, tag="lg")
            nc.vector.tensor_tensor(lg[:], ps2[:], bf2_b[:], op=mybir.AluOpType.add)
            nmx = work.tile([128, 1], _f32, tag="nmx")
            nc.vector.tensor_reduce(nmx[:], lg[:], axis=mybir.AxisListType.X,
                                    op=mybir.AluOpType.max, negate=True)
            ex = work.tile([128, D_OUT], _f32, tag="ex")
            sm = work.tile([128, 1], _f32, tag="sm")
            nc.scalar.activation(ex[:], lg[:], ACT.Exp, bias=nmx[:, :1], scale=1.0,
                                 accum_out=sm[:, :1])
            rin = work.tile([128, 1], _f32, tag="rin")
            nc.vector.reciprocal(rin[:], sm[:])
            ot = work.tile([128, D_OUT], _f32, tag="ot")
            nc.vector.tensor_scalar_mul(ot[:], ex[:], rin[:, :1])
            nc.sync.dma_start(out_d.ap()[m * 128:(m + 1) * 128, :], ot[:])

    nc.compile()
    return nc


def _run(inputs, trace=False):
    x = np.asarray(inputs["x"], dtype=np.float32)
    edge_index = np.asarray(inputs["edge_index"])
    deg = np.bincount(
        np.concatenate([edge_index[1], np.arange(N, dtype=edge_index.dtype)]),
        minlength=N,
    ).astype(np.float32)
    dinv = np.zeros(N, dtype=np.float32)
    nz = deg > 0
    dinv[nz] = (1.0 / np.sqrt(deg[nz])).astype(np.float32)

    per_core, group_sizes, TC = _prepare(x, edge_index, dinv)
    nc = _build(group_sizes, TC)

    if MODE == "bf16":
        import ml_dtypes
        mnp = ml_dtypes.bfloat16
    else:
        mnp = np.float32

    in_maps = []
    for c in range(NCORES):
        xp = np.zeros((RPAD, D), dtype=np.float32)
        xp[:RPC] = x[c * RPC:(c + 1) * RPC]
        dv = np.zeros(RPAD, dtype=np.float32)
        dv[:RPC] = dinv[c * RPC:(c + 1) * RPC]
        bb = {k: np.broadcast_to(np.asarray(inputs[k], np.float32), (128, D)).copy()
              for k in ("b1", "b2")}
        in_maps.append({
            "xT": np.ascontiguousarray(xp.T).astype(mnp),
            "dinv": dv,
            "W1": np.asarray(inputs["W1"], np.float32).astype(mnp),
            "W2": np.asarray(inputs["W2"], np.float32).astype(mnp),
            "Wf1": np.asarray(inputs["Wf1"], np.float32).astype(mnp),
            "Wf2": np.asarray(inputs["Wf2"], np.float32).astype(mnp),
            "b1": bb["b1"],
            "b2": bb["b2"],
            "bf1": np.asarray(inputs["bf1"], np.float32),
            "bf2": np.broadcast_to(np.asarray(inputs["bf2"], np.float32),
                                   (128, D_OUT)).copy(),
            "S": per_core[c]["S"],
            "idx": per_core[c]["idx"],
        })

    res = run_bass_kernel_spmd(nc, in_maps, core_ids=list(range(NCORES)),
                               trace=trace)
    out = np.concatenate([res.results[c]["out"][:RPC] for c in range(NCORES)], axis=0)
    return out, res


def kernel(**inputs):
    out, _ = _run(inputs, trace=False)
    return out
